# revision 9
# baseline (speedup 1.0000x reference)
"""Expert-parallel MoE MLP kernel for Trainium2 (8 NeuronCores, 1 expert/core).

Problem: inputs [1, 8, 16384, 512], per-expert 2-layer GELU MLP
  h   = gelu(x @ W1[e] + b1[e])      # [16384, 2048]
  out = h @ W2[e] + b2[e]            # [16384, 512]

Per-core dataflow (bf16 PE path, fp32 PSUM accumulate):
  1. x and W1/W2 are cast to bf16 on host. x blocks [512t, 128d] are
     DMA-loaded *transposed* through the xbar (2-byte dtype) straight
     into SBUF as xT [d, t] - no PE transposes at all.
  2. L1: psum[f,t] = sum_k matmul(lhsT=W1[dk, f], rhs=xT[dk, t])  (bf16, FWL)
  3. ScalarE Gelu(+b1 per-partition bias) psum -> hT sbuf [f, t] bf16
  4. L2: psum[t,d'] = sum_k matmul(lhsT=hT[fk, t], rhs=W2[fk, d']) (bf16)
     -> output lands in natural token-major layout, no output transpose
  5. DVE add b2 (broadcast) psum -> sbuf fp32, per-j 256KB DMA out so the
     tail drains as soon as each token subtile's ADD lands.

Startup schedule: DMA issue is spread across idle engine queues
(scalar: xt block0, vector: xt block1 + first w2 tiles, gpsimd: rest of
w2 + w28 + b2, sync: w1 stream) and block 0's L1 runs k-major in
f-groups of 4 so each arriving xT k-piece unlocks 16 matmuls.
"""

import os
import numpy as np
import ml_dtypes

E, C, D, F = 8, 16384, 512, 2048
P = 128
TBLK = 512  # tokens per block

FP8_PAIRS = int(os.environ.get("KERNEL_FP8_PAIRS", "1"))  # DR k-tile pairs in L2

_CACHE = {}


def _build(T, fp8_pairs, act="Gelu_apprx_tanh"):
    import concourse.mybir as mybir
    import concourse.tile as tile
    from concourse import bacc

    f32 = mybir.dt.float32
    bf16 = mybir.dt.bfloat16
    fp8 = mybir.dt.float8e4
    DR = mybir.MatmulPerfMode.DoubleRow
    gelu_fn = getattr(mybir.ActivationFunctionType, act)

    nc = bacc.Bacc("TRN2", target_bir_lowering=False, debug=False)

    NF8 = 2 * fp8_pairs          # trailing f k-tiles of L2 done in fp8
    KBF = (F // P) - NF8         # leading bf16 k-tiles of L2

    x_d = nc.dram_tensor("x", [T, D], bf16, kind="ExternalInput").ap()
    w1_d = nc.dram_tensor("w1", [D, F], bf16, kind="ExternalInput").ap()
    b1_d = nc.dram_tensor("b1", [F], f32, kind="ExternalInput").ap()
    w2_d = nc.dram_tensor("w2", [F, D], bf16, kind="ExternalInput").ap()
    if NF8:
        w28_d = nc.dram_tensor("w28", [NF8 * P, D], fp8, kind="ExternalInput").ap()
    b2_d = nc.dram_tensor("b2", [D], f32, kind="ExternalInput").ap()
    o_d = nc.dram_tensor("out", [T, D], f32, kind="ExternalOutput").ap()

    KD = D // P   # 4  k-tiles (d) for layer 1
    KF = F // P   # 16 k-tiles (f) for layer 2
    NB = T // TBLK
    JT = TBLK // P  # 4 token sub-tiles per block

    with tile.TileContext(nc) as tc:
        with (
            tc.tile_pool(name="consts", bufs=1) as consts,
            tc.tile_pool(name="xt", bufs=3) as xt_pool,
            tc.tile_pool(name="ht", bufs=2) as ht_pool,
            tc.tile_pool(name="ot", bufs=2) as ot_pool,
            tc.tile_pool(name="ph", bufs=4, space="PSUM") as ph_pool,
            tc.tile_pool(name="po", bufs=4, space="PSUM") as po_pool,
        ):
            def load_block(blk, eng=None):
                # xbar-transposed load: [TBLK, 128] DRAM -> [128, TBLK] SBUF
                eng = eng or nc.sync
                xt = xt_pool.tile([P, KD, TBLK], bf16, name="xt", tag="xt")
                t0 = blk * TBLK
                for k in range(KD):
                    eng.dma_start(
                        xt[:, k, :],
                        x_d[t0 : t0 + TBLK, k * P : (k + 1) * P],
                        transpose=True,
                    )
                return xt

            w1_r = w1_d.rearrange("(k p) f -> p k f", p=P)
            # first 4 f-tiles live in one shared tile so each k-slab of
            # them loads with a single DMA issue
            w1_abcd = consts.tile([P, KD, 4 * P], bf16)
            w1_t = [None] * 4 + [
                consts.tile([P, KD, P], bf16, name=f"w1_{f}", tag=f"w1_{f}")
                for f in range(4, KF)
            ]

            def w1sl(f, k):
                if f < 4:
                    return w1_abcd[:, k, f * P : (f + 1) * P]
                return w1_t[f][:, k, :]

            w2_r = w2_d.rearrange("(k p) d -> p k d", p=P)
            w2_t = [
                consts.tile([P, D], bf16, name=f"w2_{k}", tag=f"w2_{k}")
                for k in range(KBF)
            ]
            b1_sb = consts.tile([P, KF], f32)
            b2_bc = consts.tile([P, D], f32)
            if NF8:
                w28_sb = consts.tile([P, NF8, D], fp8)

            # --- startup issue schedule ------------------------------------
            # DMA transpose is only legal on the sync + scalar queues; both
            # are needed elsewhere, so weights ride gpsimd/vector queues.
            # gpsimd: w1 f0-f3 (one issue per k-slab), bias, then w2.
            nc.gpsimd.dma_start(w1_abcd[:, 0, :], w1_r[:, 0, 0 : 4 * P])
            # scalar: block-0 xT (critical path of the first matmul); the
            # scalar queue then naturally runs this block's activations.
            xts = {0: load_block(0, eng=nc.scalar)}
            nc.gpsimd.dma_start(b1_sb[:], b1_d.rearrange("(k p) -> p k", p=P))
            for k in range(1, KD):
                nc.gpsimd.dma_start(w1_abcd[:, k, :], w1_r[:, k, 0 : 4 * P])
            for k in range(0, KBF - 4):
                nc.gpsimd.dma_start(w2_t[k][:], w2_r[:, k, :])
            # sync: one w1 tile ahead, then block-1 xT, then the w1 stream.
            nc.sync.dma_start(w1_t[4][:], w1_r[:, :, 4 * P : 5 * P])
            if NB > 1:
                xts[1] = load_block(1, eng=nc.sync)
            for f in range(5, KF):
                nc.sync.dma_start(w1_t[f][:], w1_r[:, :, f * P : (f + 1) * P])
            # gpsimd: tail weights (needed only once L2 is deep in its k loop).
            for k in range(max(KBF - 4, 0), KBF):
                nc.gpsimd.dma_start(w2_t[k][:], w2_r[:, k, :])
            if NF8:
                nc.gpsimd.dma_start(
                    w28_sb[:], w28_d.rearrange("(k p) d -> p k d", p=P)
                )
            nc.gpsimd.dma_start(
                b2_bc[:], b2_d.unsqueeze(0).partition_broadcast(P)
            )
            # ----------------------------------------------------------------

            def activate(f, ph, hts, ht8):
                if f >= KBF:
                    nc.scalar.activation(
                        ht8[:, f - KBF, :], ph[:], gelu_fn, bias=b1_sb[:, f : f + 1]
                    )
                    hts.append(None)
                else:
                    ht_f = ht_pool.tile([P, TBLK], bf16, name=f"ht{f}", tag=f"ht{f}")
                    nc.scalar.activation(
                        ht_f[:], ph[:], gelu_fn, bias=b1_sb[:, f : f + 1]
                    )
                    hts.append(ht_f)

            def layer1(xt):
                hts = []
                ht8 = (
                    ht_pool.tile([P, NF8, TBLK], fp8, name="ht8", tag="ht8")
                    if NF8
                    else None
                )
                for f in range(KF):
                    ph = ph_pool.tile([P, TBLK], f32, name="ph", tag="ph")
                    for k in range(KD):
                        nc.tensor.matmul(
                            ph[:],
                            w1sl(f, k),
                            xt[:, k, :],
                            start=(k == 0),
                            stop=(k == KD - 1),
                        )
                    activate(f, ph, hts, ht8)
                return hts, ht8

            def layer1_kmajor(xt):
                # block 0: k-major inside f-groups of 4 so each arriving
                # xt k-piece feeds 4 psum chains at once.
                hts = []
                ht8 = (
                    ht_pool.tile([P, NF8, TBLK], fp8, name="ht8", tag="ht8")
                    if NF8
                    else None
                )
                for fg in range(KF // 4):
                    phs = [
                        ph_pool.tile([P, TBLK], f32, name="ph", tag="ph")
                        for _ in range(4)
                    ]
                    for k in range(KD):
                        for i, f in enumerate(range(fg * 4, fg * 4 + 4)):
                            nc.tensor.matmul(
                                phs[i][:],
                                w1sl(f, k),
                                xt[:, k, :],
                                start=(k == 0),
                                stop=(k == KD - 1),
                            )
                    for i, f in enumerate(range(fg * 4, fg * 4 + 4)):
                        activate(f, phs[i], hts, ht8)
                return hts, ht8

            def layer2(blk, hts, ht8):
                t0 = blk * TBLK
                ot = ot_pool.tile([P, JT, D], f32, name="ot", tag="ot")
                for j in range(JT):
                    po = po_pool.tile([P, D], f32)
                    for k in range(KBF):
                        nc.tensor.matmul(
                            po[:],
                            hts[k][:, j * P : (j + 1) * P],
                            w2_t[k][:],
                            start=(k == 0),
                            stop=(k == KBF - 1 and not NF8),
                        )
                    for p8 in range(fp8_pairs):
                        nc.tensor.matmul(
                            po[:],
                            ht8[:, 2 * p8 : 2 * p8 + 2, j * P : (j + 1) * P],
                            w28_sb[:, 2 * p8 : 2 * p8 + 2, :],
                            start=False,
                            stop=(p8 == fp8_pairs - 1),
                            perf_mode=DR,
                            skip_group_check=True,
                        )
                    nc.vector.tensor_add(ot[:, j, :], po[:], b2_bc[:])
                    nc.sync.dma_start(
                        o_d[t0 + j * P : t0 + (j + 1) * P, :], ot[:, j, :]
                    )

            for blk in range(NB):
                if blk + 2 < NB:
                    xts[blk + 2] = load_block(blk + 2)
                xt = xts.pop(blk)
                hts, ht8 = layer1_kmajor(xt) if blk == 0 else layer1(xt)
                layer2(blk, hts, ht8)

    nc.compile()
    return nc


def _get_nc(T):
    key = (T, FP8_PAIRS)
    if key not in _CACHE:
        _CACHE[key] = _build(T, FP8_PAIRS)
    return _CACHE[key]


def kernel(inputs, W1, b1, W2, b2):
    from concourse.bass_utils import run_bass_kernel_spmd

    x16 = np.ascontiguousarray(
        np.asarray(inputs, dtype=np.float32)[0].astype(ml_dtypes.bfloat16)
    )
    W1_16 = np.ascontiguousarray(
        np.asarray(W1, dtype=np.float32).astype(ml_dtypes.bfloat16)
    )
    W2_16 = np.ascontiguousarray(
        np.asarray(W2, dtype=np.float32).astype(ml_dtypes.bfloat16)
    )
    NF8 = 2 * FP8_PAIRS
    if NF8:
        W2_8 = np.ascontiguousarray(
            np.asarray(W2, dtype=np.float32)[:, F - NF8 * 128 :, :].astype(
                ml_dtypes.float8_e4m3fn
            )
        )
    b1 = np.ascontiguousarray(np.asarray(b1, dtype=np.float32))
    b2 = np.ascontiguousarray(np.asarray(b2, dtype=np.float32))

    nc = _get_nc(C)
    in_maps = []
    for e in range(E):
        m = {
            "x": x16[e],
            "w1": W1_16[e],
            "b1": b1[e],
            "w2": W2_16[e],
            "b2": b2[e],
        }
        if NF8:
            m["w28"] = W2_8[e]
        in_maps.append(m)
    trace = os.environ.get("KERNEL_TRACE", "0") == "1"
    res = run_bass_kernel_spmd(
        nc, in_maps, core_ids=list(range(E)), trace=trace
    )
    if trace:
        kernel.last_exec_time_ns = res.exec_time_ns
    out = np.stack([res.results[e]["out"] for e in range(E)], axis=0)[None]
    return out
